# revision 36
# baseline (speedup 1.0000x reference)
"""nn_PatchMerging3D Trainium2 kernel.

Full inputs: x (2, 96, 32, 128, 128) f32, w (192, 768), gamma (768), beta (768).
Output: (2, 192, 16, 64, 64) f32.

Sharding: D2 (=16) split across 8 cores, 2 d2-planes per core, both batches.
Per-core slab: x[:, :, 4k:4k+4, :, :].

Per-core kernel (bass/Tile), ~157 us/iter on HW (~400 GB/s/core, at the
read+write HBM roofline; baseline was 342 us):
  * SBUF X layout: partition p = dd*64 + hh*32 + c32 (c = 32g + c32),
    free = g*GF + h2*128 + w. 12 HWDGE DMAs per 4-tile group (one per
    (dd, hh, g)), 3-dim APs, 512B contiguous runs, disjoint 32-partition
    groups so all 16 SDMA ports run concurrently. xin bufs=3.
  * LayerNorm folded into the 768x192 projection algebraically:
      wp = gamma*w ; M = xf @ wp.T   (PE, 6 K=128 chunks, float32r)
      S1 = sum xf (ones column in lhsT, psA row 96)
      S2 = sum xf^2 (ACT Square -> bf16 XQ, ones-lhsT bf16 matmuls into
           psB2 row 96 -- shares psB's PSUM bank via tile_position; bf16
           sidesteps the f32r dst-partition-0 ISA restriction)
      var = S2/768 - mu^2 (musq on ACT in parallel with mu on DVE);
      sig = ACT Sqrt(var + eps); rsig = DVE reciprocal
      y = rsig * (M - mu x s [+ t x sig])  -- rank-1 terms accumulated in
      PSUM by the PE; rsig broadcast across partitions via a ones(1,96)
      matmul into psR (bufs=2), staged to SBUF (DVE reads one PSUM
      operand max), final scale on DVE.
  * DMA-queue separation is the key scheduling trick: Tile round-robins
    HW DMAs over 8 FIFO lanes in emission order, so a late output DMA
    would block next-group input DMAs sharing its lane. Inputs dispatch
    on SP.SEQ/HWDGE lanes; outputs go through the Pool/SWDGE path
    (separate sequencer AND separate lanes).
  * Timing builds amortize the For_i all-engine barrier (a ~55 us
    pipeline flush) over inner=4 full passes per loop iteration.
"""

import os
import sys

for _p in ("/opt/trn_rl_repo", "/root/.axon_site/_ro/trn_rl_repo"):
    if os.path.isdir(_p) and _p not in sys.path:
        sys.path.insert(0, _p)

import numpy as np

import concourse.bacc as bacc
import concourse.mybir as mybir
from concourse.tile import TileContext
from concourse.bass_utils import run_bass_kernel_spmd

F32 = mybir.dt.float32
F32R = mybir.dt.float32r
BF16 = mybir.dt.bfloat16
AF = mybir.ActivationFunctionType
OP = mybir.AluOpType

C = 96
W = 128
W2 = 64
H2T = 8          # h2 values per position tile
NPOS = 512       # positions per tile
KCH = 6          # contraction chunks (g, ww)
GJ = 4           # position tiles per DMA group
LN_EPS = 1e-5
NCORES = 8
STAGES = "full"      # "mains" = DMA + main matmuls + evict only (perf probe)
MERGED_DMA = False   # 4 input DMAs per group needs 4-dim APs; HW caps at 3
OUT_DMA_ACT = True   # output DMAs dispatched from the ACT sequencer
XIN_BUFS = 3

# full-problem per-core loop counts
NB, ND, NJ = 2, 2, 8


def _host_prep(w, gamma, beta):
    w = np.asarray(w, np.float32)
    gamma = np.asarray(gamma, np.float32)
    beta = np.asarray(beta, np.float32)
    wp = w * gamma[None, :]
    s = wp.sum(axis=1)
    t = (w * beta[None, :]).sum(axis=1)

    dd = np.arange(2)[:, None, None]
    hh = np.arange(2)[None, :, None]
    ii = np.arange(32)[None, None, :]
    wA = np.zeros((KCH, 128, 97), np.float32)
    wB = np.zeros((KCH, 128, 96), np.float32)
    for g in range(3):
        for ww in range(2):
            q = g * 2 + ww
            cf = (dd * 384 + hh * 192 + ww * 96 + 32 * g + ii).reshape(128)
            wA[q, :, :96] = wp[0:96, cf].T
            wA[q, :, 96] = 1.0
            wB[q, :, :] = wp[96:192, cf].T
    return {
        "wA": wA,
        "wB": wB,
        "negs": (-s).reshape(1, 192).astype(np.float32),
        "tvec": t.reshape(1, 192).astype(np.float32),
        "ones_row": np.ones((1, 96), np.float32),
    }, bool(np.any(beta != 0.0))


def _tile_body(nc, y, b, dL, j, jl, X, GF,
               wA_sb, wB_sb, negs_sb, tvec_sb, ones_sb, onesc_sb, ebias,
               xsq_pool, y_pool, small_pool, rep_pool, psum_pool, psum_r_pool,
               has_beta):
    h0 = H2T * jl
    if STAGES != "mains":
        XQ = xsq_pool.tile([128, 3072], BF16)
        for g in range(3):
            nc.scalar.activation(
                XQ[:, g * 1024:(g + 1) * 1024],
                X[:, g * GF + h0 * 128: g * GF + (h0 + H2T) * 128].bitcast(F32),
                AF.Square)

    psA = psum_pool.tile([97, NPOS], F32)
    psB2 = psum_pool.tile([97, NPOS], F32)
    psB = psB2[0:96, :]
    psS = psB2[96:97, :]
    wA_v = wA_sb[:].rearrange("p (q m) -> p q m", q=KCH)
    wB_v = wB_sb[:].rearrange("p (q m) -> p q m", q=KCH)
    Xv = X[:].rearrange("p (g h2g w2 ww) -> p g h2g w2 ww",
                        g=3, h2g=H2T * GJ, w2=W2, ww=2)
    XQv = (None if STAGES == "mains" else
           XQ[:].rearrange("p (g h2 w2 ww) -> p g h2 w2 ww",
                           g=3, h2=H2T, w2=W2, ww=2))
    for g in range(3):
        for ww in range(2):
            q = g * 2 + ww
            rhs = Xv[:, g, h0:h0 + H2T, :, ww]
            nc.tensor.matmul(psA[:], wA_v[:, q], rhs,
                             start=(q == 0), stop=(q == KCH - 1))
            nc.tensor.matmul(psB, wB_v[:, q], rhs,
                             start=(q == 0), stop=(q == KCH - 1))
            if STAGES != "mains":
                nc.tensor.matmul(psS, onesc_sb[:],
                                 XQv[:, g, :, :, ww],
                                 start=(q == 0), stop=(q == KCH - 1),
                                 tile_position=(0, 96), skip_group_check=True)

    yt = y_pool.tile([96, 2 * NPOS], F32)
    if STAGES == "mains":
        nc.vector.tensor_copy(yt[:, 0:NPOS], psA[0:96, :])
        nc.vector.tensor_copy(yt[:, NPOS:], psB)
    else:
        # stats: mu for the rank-1 PE correction, rsig via ACT Rsqrt
        mu_t = small_pool.tile([1, NPOS], F32R)
        vm = small_pool.tile([1, 2 * NPOS], F32)
        sr = small_pool.tile([1, 2 * NPOS], F32R)
        mu = mu_t[:]
        sig = sr[:, 0:NPOS]
        rsig = sr[:, NPOS:]
        var = vm[:, 0:NPOS]
        musq = vm[:, NPOS:]
        # mu on DVE (feeds the PE rank-1 rhs); musq on ACT in parallel
        nc.vector.tensor_scalar(mu, psA[96:97, :], 1.0 / 768.0, None, OP.mult)
        nc.scalar.activation(musq, psA[96:97, :], AF.Square, scale=1.0 / 768.0)
        # var = S2/768 - mu^2   (+eps folded into the sqrt bias)
        nc.vector.scalar_tensor_tensor(var, psS, 1.0 / 768.0, musq,
                                       OP.mult, OP.subtract)
        nc.scalar.activation(sig, var, AF.Sqrt, bias=ebias[:])
        with nc.allow_low_precision(reason="f32r rsig for matmul broadcast"):
            nc.vector.reciprocal(rsig, sig.bitcast(F32))

        # rank-1 corrections (main group closed; HW has_written accumulates)
        nc.tensor.matmul(psA[0:96, :], negs_sb[0:1, 0:96], mu,
                         start=False, stop=True, skip_group_check=True)
        nc.tensor.matmul(psB, negs_sb[0:1, 96:192], mu,
                         start=False, stop=True, skip_group_check=True)
        if has_beta:
            # sig2 via DVE (ACT outputs may not feed f32r matmuls directly);
            # reuses var's slot, which is dead once sig/rsig exist.
            sig2 = vm[:, 0:NPOS].bitcast(F32R)
            with nc.allow_low_precision(reason="f32r sig for matmul rhs"):
                nc.vector.reciprocal(sig2, rsig.bitcast(F32))
            nc.tensor.matmul(psA[0:96, :], tvec_sb[0:1, 0:96], sig2,
                             start=False, stop=True, skip_group_check=True)
            nc.tensor.matmul(psB, tvec_sb[0:1, 96:192], sig2,
                             start=False, stop=True, skip_group_check=True)

        # rsig broadcast across partitions (PE ones-matmul); DVE cannot read
        # two PSUM operands, so stage psR through SBUF (rep) first.
        psR = psum_r_pool.tile([96, NPOS], F32)
        rep = rep_pool.tile([96, NPOS], F32)
        nc.tensor.matmul(psR[:], ones_sb[:], rsig, start=True, stop=True)
        nc.vector.tensor_copy(rep[:], psR[:])
        nc.vector.tensor_tensor(yt[:, 0:NPOS], psA[0:96, :], rep[:], OP.mult)
        nc.vector.tensor_tensor(yt[:, NPOS:], psB, rep[:], OP.mult)

    # Output on the Pool/SWDGE path: separate sequencer AND separate DMA
    # lanes, so a late output never FIFO-blocks next-group input DMAs.
    nc.gpsimd.dma_start(y[b, dL, j],
                        yt[:].rearrange("p (half f) -> p half f", half=2))


def build_kernel(nc, reps=1, has_beta=True):
    x = nc.dram_tensor("x", [NB, C, 2 * ND, 16 * NJ, W], F32,
                       kind="ExternalInput")
    wA_d = nc.dram_tensor("wA", [KCH, 128, 97], F32, kind="ExternalInput")
    wB_d = nc.dram_tensor("wB", [KCH, 128, 96], F32, kind="ExternalInput")
    negs_d = nc.dram_tensor("negs", [1, 192], F32, kind="ExternalInput")
    tvec_d = nc.dram_tensor("tvec", [1, 192], F32, kind="ExternalInput")
    ones_d = nc.dram_tensor("ones_row", [1, 96], F32, kind="ExternalInput")
    y = nc.dram_tensor("y", [NB, ND, NJ, 96, 2, NPOS], F32,
                       kind="ExternalOutput")

    GF = GJ * 1024
    with TileContext(nc) as tc:
        with (
            tc.tile_pool(name="wpool", bufs=1) as wpool,
            tc.tile_pool(name="xin", bufs=XIN_BUFS) as xin_pool,
            tc.tile_pool(name="xsq", bufs=2) as xsq_pool,
            tc.tile_pool(name="yout", bufs=4) as y_pool,
            tc.tile_pool(name="small", bufs=2) as small_pool,
            tc.tile_pool(name="rep", bufs=3) as rep_pool,
            tc.tile_pool(name="psAB", bufs=3, space="PSUM") as psum_pool,
            tc.tile_pool(name="psR", bufs=2, space="PSUM") as psum_r_pool,
        ):
            wA_sb = wpool.tile([128, KCH * 97], F32R)
            wB_sb = wpool.tile([128, KCH * 96], F32R)
            negs_sb = wpool.tile([1, 192], F32R)
            tvec_sb = wpool.tile([1, 192], F32R)
            ones_sb = wpool.tile([1, 96], F32R)
            onesc_sb = wpool.tile([128, 1], BF16)
            ebias = wpool.tile([1, 1], F32)
            nc.vector.memset(ebias[:], LN_EPS)
            nc.vector.memset(onesc_sb[:], 1.0)
            nc.sync.dma_start(
                wA_sb[:].rearrange("p (q m) -> p q m", q=KCH),
                wA_d[:].rearrange("q p m -> p q m").bitcast(F32R))
            nc.sync.dma_start(
                wB_sb[:].rearrange("p (q m) -> p q m", q=KCH),
                wB_d[:].rearrange("q p m -> p q m").bitcast(F32R))
            nc.sync.dma_start(negs_sb[:], negs_d[:].bitcast(F32R))
            nc.sync.dma_start(tvec_sb[:], tvec_d[:].bitcast(F32R))
            nc.sync.dma_start(ones_sb[:], ones_d[:].bitcast(F32R))

            inner = 4 if (reps > 1 and reps % 4 == 0) else (2 if (reps > 1 and reps % 2 == 0) else 1)
            if reps > 1:
                import concourse.mybir as _mb
                loop_cm = tc.For_i(0, reps // inner, 1,
                                   hint_engines=(_mb.EngineType.PE,
                                                 _mb.EngineType.SP,
                                                 _mb.EngineType.DVE,
                                                 _mb.EngineType.Activation,
                                                 _mb.EngineType.Pool))
            else:
                import contextlib
                loop_cm = contextlib.nullcontext()
            with loop_cm:
              for _rep in range(inner):
                for b in range(NB):
                    for dL in range(ND):
                        for jj in range(NJ // GJ):
                            X = xin_pool.tile([128, 3 * GF], F32R)
                            for dd in range(2):
                                for hh in range(2):
                                    p0 = dd * 64 + hh * 32
                                    if MERGED_DMA:
                                        src = x[b, :, 2 * dL + dd,
                                                16 * GJ * jj + hh:
                                                16 * GJ * (jj + 1): 2, :]
                                        src = src.rearrange(
                                            "(g c) h w -> c g h w", g=3)
                                        dst = X[p0:p0 + 32, :].rearrange(
                                            "p (g h w) -> p g h w",
                                            g=3, h=16 * GJ // 2)
                                        nc.sync.dma_start(
                                            dst, src.bitcast(F32R))
                                    else:
                                        for g in range(3):
                                            src = x[b, 32 * g:32 * g + 32,
                                                    2 * dL + dd,
                                                    16 * GJ * jj + hh:
                                                    16 * GJ * (jj + 1): 2, :]
                                            nc.sync.dma_start(
                                                X[p0:p0 + 32,
                                                  g * GF:(g + 1) * GF],
                                                src.bitcast(F32R))
                            for jl in range(GJ):
                                _tile_body(nc, y, b, dL, GJ * jj + jl, jl,
                                           X, GF,
                                           wA_sb, wB_sb, negs_sb, tvec_sb,
                                           ones_sb, onesc_sb, ebias,
                                           xsq_pool, y_pool, small_pool,
                                           rep_pool, psum_pool, psum_r_pool,
                                           has_beta)
    nc.compile()
    return nc


_NC_CACHE = {}


def _get_nc(reps, has_beta):
    key = (reps, has_beta)
    if key not in _NC_CACHE:
        nc = bacc.Bacc("TRN2", target_bir_lowering=False)
        build_kernel(nc, reps=reps, has_beta=has_beta)
        _NC_CACHE[key] = nc
    return _NC_CACHE[key]


def _decode_y(y_raw):
    """(NB, ND, NJ, 96, 2, 512) -> (NB, 192, ND, 8*NJ, 64)"""
    z = y_raw.reshape(NB, ND, NJ, 96, 2, H2T, W2)
    z = z.transpose(0, 4, 3, 1, 2, 5, 6)
    return z.reshape(NB, 192, ND, NJ * H2T, W2)


def run_cores(x, w, gamma, beta, reps=1):
    """Run the SPMD kernel; returns full output (2, 192, 16, 64, 64)."""
    x = np.asarray(x, np.float32)
    prep, has_beta = _host_prep(w, gamma, beta)
    nc = _get_nc(reps, has_beta)
    in_maps = []
    for k in range(NCORES):
        m = {"x": np.ascontiguousarray(x[:, :, 4 * k:4 * k + 4, :, :])}
        m.update(prep)
        in_maps.append(m)
    res = run_bass_kernel_spmd(nc, in_maps, core_ids=list(range(NCORES)))
    out = np.empty((2, 192, 16, 64, 64), np.float32)
    for k in range(NCORES):
        out[:, :, 2 * k:2 * k + 2] = _decode_y(res.results[k]["y"])
    return out


def kernel(x, w, gamma, beta):
    return run_cores(x, w, gamma, beta, reps=1)
